# revision 29
# baseline (speedup 1.0000x reference)
"""Chamfer distance loss kernel for Trainium2 (8 NeuronCores).

Strategy
--------
reference: D[i,j] = ||pred_i - gt_j||^2 ; out = mean_i min_j D + mean_j min_i D.

We decompose into 8 independent jobs (4 batches x 2 directions), one per core.
For one job (query set A, candidate set B, both of size N=8192):

  * Host sorts A and B by x-coordinate.  For query rank i, the true nearest
    neighbor is almost always within a small rank window of i in the sorted
    B order.  Each 128-row query tile t scans the candidate window
    [128t - WL, 128t + SPAN - WL) (clamped via padding), SPAN wide.
  * The kernel computes, per query row, max_j (2<a,b_j> - ||b_j||^2) over the
    window via a K=4 TensorE matmul (features [2ax,2ay,2az,1] x [bx,by,bz,-||b||^2])
    and a VectorE free-axis max-reduce.  Then
    min_j D = ||a||^2 - rowmax, computed on host.
  * Exactness certificate (host): for query i with window [lo, hi), any
    excluded candidate j < lo has |a_x - b_x[j]| >= a_x - b_x[lo-1], so if
    band_min <= (x-margin)^2 on both sides the band min is the true min.
    The few rows that fail the certificate get an exact host-side scan.

Cores: core = 2*batch + direction (0: pred->gt, 1: gt->pred).
"""

import os

import numpy as np

import concourse.bass as bass
import concourse.tile as tile
from concourse import bacc, mybir
from concourse.bass_utils import run_bass_kernel_spmd

N = 8192  # points per cloud (both pred and gt)
B = 4  # batches
ROWT = 128  # query rows per tile
NTILES = N // ROWT  # 64
SPAN = 320  # candidate window width per row tile
WL = 96  # left extension of the window
WR = SPAN - WL - ROWT  # 192: right extension
PADDED = WL + N + WR  # padded candidate count
PAD_COORD = 1000.0  # sentinel coordinate for padding (never wins a min)

_CACHE = {}

# test.py introspection: set to BassKernelResults of the last run
LAST_RESULTS = None


NGROUP = NTILES // 4  # 16 row-tiles per PE row group


def _build_program():
    nc = bacc.Bacc(
        "TRN2", target_bir_lowering=False, debug=False, num_devices=8
    )
    # Row group j (PE rows 32j..32j+4, via tile_position) handles row-tiles
    # t = 4g + j.  Both matmul operands must live at partitions 32j..32j+4,
    # so the host ships per-group gathered slices: group j's query columns
    # [4, 16*128] and its candidate windows [4, 16*SPAN] — no duplicated
    # bytes.  Four matmuls (one per group) run concurrently on the PE.
    # Matmul operands must start at partition 32j (PE row-group base), so we
    # use K=32 matmuls over the full 32-partition strip of row group j:
    #   - c_sb (moving) is interleaved: partition 32j + 4m + f = feature f,
    #     row group j, column-chunk m (chunk m covers reduce-groups 2m,2m+1).
    #     Its DMA writes all 128 partitions -> full 16-port bandwidth.
    #   - q_stat (stationary) is zero-padded per reduce-group: the [32, 128]
    #     weight slice for (g, j) has query features only in rows 4*(g//2)..+4
    #     and zeros elsewhere, so the other 7 chunks in the moving strip are
    #     multiplied away exactly.
    CCH = 2 * SPAN  # c columns per chunk
    qfeat_d = nc.declare_dram_parameter(
        "qfeat", [128, NGROUP * ROWT], mybir.dt.float32, isOutput=False
    )
    cfeat_d = nc.declare_dram_parameter(
        "cfeat", [128, CCH], mybir.dt.float32, isOutput=False
    )
    rowmax_out = nc.declare_dram_parameter(
        "rowmax", [ROWT, NTILES], mybir.dt.float32, isOutput=True
    )

    with tile.TileContext(nc) as tc:
        with (
            tc.tile_pool(name="feats", bufs=1) as feats,
            tc.tile_pool(name="psum", bufs=2, space="PSUM") as psum_pool,
            tc.tile_pool(name="outp", bufs=1) as outp,
        ):
            # Separate tiles per input chunk so Tile's dependency tracking
            # lets early matmuls start while later chunks are still in
            # flight.  c splits by parity column (r), q by g-pairs.
            c_sbs = [
                feats.tile([128, SPAN], mybir.dt.float32, tag=f"c{r}", name=f"c{r}")
                for r in range(2)
            ]
            QQ = 2 * ROWT
            q_sbs = [
                feats.tile([128, QQ], mybir.dt.float32, tag=f"q{i}", name=f"q{i}")
                for i in range(8)
            ]
            # First matmul needs only c0 + q pair 0 — land those first, one
            # per HWDGE queue, then stream the rest in consumption order.
            nc.sync.dma_start(out=c_sbs[0][:], in_=cfeat_d[:, :SPAN])
            nc.scalar.dma_start(out=q_sbs[0][:], in_=qfeat_d[:, :QQ])
            nc.scalar.dma_start(out=q_sbs[1][:], in_=qfeat_d[:, QQ : 2 * QQ])
            nc.sync.dma_start(out=q_sbs[2][:], in_=qfeat_d[:, 2 * QQ : 3 * QQ])
            nc.scalar.dma_start(out=q_sbs[3][:], in_=qfeat_d[:, 3 * QQ : 4 * QQ])
            nc.sync.dma_start(out=c_sbs[1][:], in_=cfeat_d[:, SPAN:])
            nc.scalar.dma_start(out=q_sbs[4][:], in_=qfeat_d[:, 4 * QQ : 5 * QQ])
            nc.sync.dma_start(out=q_sbs[5][:], in_=qfeat_d[:, 5 * QQ : 6 * QQ])
            nc.scalar.dma_start(out=q_sbs[6][:], in_=qfeat_d[:, 6 * QQ : 7 * QQ])
            nc.sync.dma_start(out=q_sbs[7][:], in_=qfeat_d[:, 7 * QQ :])

            rmax = outp.tile([ROWT, NTILES], mybir.dt.float32)

            # 4 row-tiles share one 4-bank PSUM tensor; a single
            # TENSOR_REDUCE with a 3D AP [128, 4, SPAN] reduces all 4
            # (out free size 4), amortizing the per-op PSUM overhead.
            # Even reduce-groups first: they only need c chunk r=0.
            g_order = list(range(0, NGROUP, 2)) + list(range(1, NGROUP, 2))
            for g in g_order:
                r = g % 2
                q_sb = q_sbs[g // 2]
                qcol = ROWT * (g % 2)
                ps = psum_pool.tile(
                    [ROWT, 4, 512], mybir.dt.float32, tag="ps", name=f"ps{g}"
                )
                for j in range(4):
                    p0 = 32 * j
                    nc.tensor.matmul(
                        ps[:, j, :SPAN],
                        lhsT=q_sb[p0 : p0 + 32, qcol : qcol + ROWT],
                        rhs=c_sbs[r][p0 : p0 + 32, :],
                        start=True,
                        stop=True,
                        tile_position=(32 * j, 0),
                    )
                nc.vector.reduce_max(
                    rmax[:, 4 * g : 4 * g + 4],
                    ps[:, :, :SPAN],
                    axis=mybir.AxisListType.X,
                )

            # Stream the output in two halves (even-g columns finish first
            # under g_order) so the exit drain has less left to wait for.
            rm_v = rmax.rearrange("p (g c) -> p g c", c=4)
            ro_v = rowmax_out.rearrange("p (g c) -> p g c", c=4)
            nc.scalar.dma_start(
                out=ro_v[:, 0:NGROUP:2, :], in_=rm_v[:, 0:NGROUP:2, :]
            )
            nc.sync.dma_start(
                out=ro_v[:, 1:NGROUP:2, :], in_=rm_v[:, 1:NGROUP:2, :]
            )
    nc.compile()
    return nc


def _job_arrays(A, Bset):
    """Build per-row-group gathered feature arrays for one job."""
    ao = np.argsort(A[:, 0], kind="stable")
    bo = np.argsort(Bset[:, 0], kind="stable")
    As = np.ascontiguousarray(A[ao])
    Bs = np.ascontiguousarray(Bset[bo])

    qfeat = np.empty((4, N), np.float32)
    qfeat[0:3] = (2.0 * As).T
    qfeat[3] = 1.0

    cfeat = np.empty((4, PADDED), np.float32)
    cfeat[0:3] = PAD_COORD
    cfeat[3] = -3.0 * PAD_COORD * PAD_COORD
    cfeat[0:3, WL : WL + N] = Bs.T
    cfeat[3, WL : WL + N] = -(Bs.astype(np.float64) ** 2).sum(1).astype(np.float32)

    # c_big interleaved: partition 32j + 4m + f = (feature f, row group j,
    # chunk m), chunk m covering reduce-groups {2m, 2m+1}.
    # q_stat zero-padded stationary: for reduce-group g, row group j, the
    # [32, 128] slice at columns 128g has features only in rows 4*(g//2)..+4.
    q_stat = np.zeros((128, NGROUP * ROWT), np.float32)
    c_big = np.empty((128, 2 * SPAN), np.float32)
    g = np.arange(NGROUP)
    for j in range(4):
        t = 4 * g + j
        cidx = (ROWT * t)[:, None] + np.arange(SPAN)[None, :]
        cj = cfeat[:, cidx]  # [4f, 16g, SPAN]
        c_big[32 * j : 32 * j + 32] = (
            cj.reshape(4, 8, 2 * SPAN).transpose(1, 0, 2).reshape(32, 2 * SPAN)
        )
        for gg in range(NGROUP):
            tt = 4 * gg + j
            m = gg // 2
            q_stat[
                32 * j + 4 * m : 32 * j + 4 * m + 4,
                ROWT * gg : ROWT * gg + ROWT,
            ] = qfeat[:, ROWT * tt : ROWT * tt + ROWT]
    in_map = {"qfeat": q_stat, "cfeat": c_big}
    return As, Bs, in_map


def kernel(pred: np.ndarray, gt: np.ndarray) -> np.ndarray:
    global LAST_RESULTS
    pred = np.asarray(pred, dtype=np.float32)
    gt = np.asarray(gt, dtype=np.float32)
    assert pred.shape == (B, N, 3) and gt.shape == (B, N, 3)

    if "nc" not in _CACHE:
        _CACHE["nc"] = _build_program()
    nc = _CACHE["nc"]

    jobs = []
    in_maps = []
    for b in range(B):
        for A, Bset in ((pred[b], gt[b]), (gt[b], pred[b])):
            As, Bs, in_map = _job_arrays(A, Bset)
            jobs.append((As, Bs))
            in_maps.append(in_map)

    trace = bool(int(os.environ.get("CHAMFER_TRACE", "0")))
    bk = run_bass_kernel_spmd(nc, in_maps, list(range(8)), trace=trace)
    LAST_RESULTS = bk
    results = bk.results

    # Host: undo the rowmax formulation, certify, fix up, and average.
    total = 0.0
    i = np.arange(N)
    t = i // ROWT
    lo = ROWT * t - WL  # window start (unpadded coords, may be < 0)
    hi = ROWT * t + (SPAN - WL)  # window end (may be > N)
    for (As, Bs), r in zip(jobs, results):
        rowmax = np.asarray(r["rowmax"])  # [128, 64]
        asq = (As.astype(np.float64) ** 2).sum(1)
        d_band = asq - rowmax.T.reshape(-1).astype(np.float64)

        bx = Bs[:, 0].astype(np.float64)
        ax = As[:, 0].astype(np.float64)
        lmarg = np.where(lo >= 1, ax - bx[np.clip(lo - 1, 0, N - 1)], np.inf)
        rmarg = np.where(hi < N, bx[np.clip(hi, 0, N - 1)] - ax, np.inf)
        marg = np.minimum(lmarg, rmarg)
        ok = (marg >= 0) & (d_band <= marg * marg)
        bad = np.flatnonzero(~ok)
        if bad.size:
            Ad = As[bad].astype(np.float64)
            Bd = Bs.astype(np.float64)
            d = ((Ad[:, None, :] - Bd[None, :, :]) ** 2).sum(-1)
            d_band[bad] = d.min(1)
        total += d_band.mean()

    return np.float32(total / B)


# revision 33
# speedup vs baseline: 1.0113x; 1.0113x over previous
"""Chamfer distance loss kernel for Trainium2 (8 NeuronCores).

Strategy
--------
reference: D[i,j] = ||pred_i - gt_j||^2 ; out = mean_i min_j D + mean_j min_i D.

We decompose into 8 independent jobs (4 batches x 2 directions), one per core.
For one job (query set A, candidate set B, both of size N=8192):

  * Host sorts A and B by x-coordinate.  For query rank i, the true nearest
    neighbor is almost always within a small rank window of i in the sorted
    B order.  Each 128-row query tile t scans the candidate window
    [128t - WL, 128t + SPAN - WL) (clamped via padding), SPAN wide.
  * The kernel computes, per query row, max_j (2<a,b_j> - ||b_j||^2) over the
    window via a K=4 TensorE matmul (features [2ax,2ay,2az,1] x [bx,by,bz,-||b||^2])
    and a VectorE free-axis max-reduce.  Then
    min_j D = ||a||^2 - rowmax, computed on host.
  * Exactness certificate (host): for query i with window [lo, hi), any
    excluded candidate j < lo has |a_x - b_x[j]| >= a_x - b_x[lo-1], so if
    band_min <= (x-margin)^2 on both sides the band min is the true min.
    The few rows that fail the certificate get an exact host-side scan.

Cores: core = 2*batch + direction (0: pred->gt, 1: gt->pred).
"""

import os

import numpy as np

import concourse.bass as bass
import concourse.tile as tile
from concourse import bacc, mybir
from concourse.bass_utils import run_bass_kernel_spmd

N = 8192  # points per cloud (both pred and gt)
B = 4  # batches
ROWT = 128  # query rows per tile
NTILES = N // ROWT  # 64
SPAN = 320  # candidate window width per row tile
WL = 96  # left extension of the window
WR = SPAN - WL - ROWT  # 192: right extension
PADDED = WL + N + WR  # padded candidate count
PAD_COORD = 1000.0  # sentinel coordinate for padding (never wins a min)

_CACHE = {}

# test.py introspection: set to BassKernelResults of the last run
LAST_RESULTS = None


NGROUP = NTILES // 4  # 16 row-tiles per PE row group


def _build_program():
    nc = bacc.Bacc(
        "TRN2", target_bir_lowering=False, debug=False, num_devices=8
    )
    # Row group j (PE rows 32j..32j+4, via tile_position) handles row-tiles
    # t = 4g + j.  Both matmul operands must live at partitions 32j..32j+4,
    # so the host ships per-group gathered slices: group j's query columns
    # [4, 16*128] and its candidate windows [4, 16*SPAN] — no duplicated
    # bytes.  Four matmuls (one per group) run concurrently on the PE.
    # Matmul operands must start at partition 32j (PE row-group base), so we
    # use K=32 matmuls over the full 32-partition strip of row group j:
    #   - c_sb (moving) is interleaved: partition 32j + 4m + f = feature f,
    #     row group j, column-chunk m (chunk m covers reduce-groups 2m,2m+1).
    #     Its DMA writes all 128 partitions -> full 16-port bandwidth.
    #   - q_stat (stationary) is zero-padded per reduce-group: the [32, 128]
    #     weight slice for (g, j) has query features only in rows 4*(g//2)..+4
    #     and zeros elsewhere, so the other 7 chunks in the moving strip are
    #     multiplied away exactly.
    CCH = 2 * SPAN  # c columns per chunk
    qfeat_d = nc.declare_dram_parameter(
        "qfeat", [128, NGROUP * ROWT], mybir.dt.float32, isOutput=False
    )
    cfeat_d = nc.declare_dram_parameter(
        "cfeat", [128, CCH], mybir.dt.float32, isOutput=False
    )
    rowmax_out = nc.declare_dram_parameter(
        "rowmax", [ROWT, NTILES], mybir.dt.float32, isOutput=True
    )

    with tile.TileContext(nc) as tc:
        with (
            tc.tile_pool(name="feats", bufs=1) as feats,
            tc.tile_pool(name="psum", bufs=2, space="PSUM") as psum_pool,
            tc.tile_pool(name="outp", bufs=1) as outp,
        ):
            # Separate tiles per input chunk so Tile's dependency tracking
            # lets early matmuls start while later chunks are still in
            # flight.  c splits by parity column (r); q: small leading
            # chunks for a fast start, bigger trailing ones to keep the
            # tile/semaphore count (and the exit-drain tail) low.
            c_sbs = [
                feats.tile([128, SPAN], mybir.dt.float32, tag=f"c{r}", name=f"c{r}")
                for r in range(2)
            ]
            QQ = 2 * ROWT
            # columns (in units of QQ): tile -> q chunk bounds
            q_bounds = [(0, 1), (1, 2), (2, 4), (4, 6), (6, 8)]
            q_sbs = [
                feats.tile(
                    [128, (b - a) * QQ],
                    mybir.dt.float32,
                    tag=f"q{i}",
                    name=f"q{i}",
                )
                for i, (a, b) in enumerate(q_bounds)
            ]
            # First matmuls need c0 + q chunk 0; the scalar queue spins up
            # ~3us earlier than sync, so the critical chunks go there.
            nc.scalar.dma_start(out=c_sbs[0][:], in_=cfeat_d[:, :SPAN])
            nc.sync.dma_start(out=q_sbs[0][:], in_=qfeat_d[:, :QQ])
            nc.scalar.dma_start(out=q_sbs[1][:], in_=qfeat_d[:, QQ : 2 * QQ])
            nc.sync.dma_start(out=q_sbs[2][:], in_=qfeat_d[:, 2 * QQ : 4 * QQ])
            nc.scalar.dma_start(out=c_sbs[1][:], in_=cfeat_d[:, SPAN:])
            nc.sync.dma_start(out=q_sbs[3][:], in_=qfeat_d[:, 4 * QQ : 6 * QQ])
            nc.scalar.dma_start(out=q_sbs[4][:], in_=qfeat_d[:, 6 * QQ :])

            def q_slice(g):
                for i, (a, b) in enumerate(q_bounds):
                    if a <= g // 2 < b:
                        return q_sbs[i], ROWT * (g - 2 * a)
                raise AssertionError

            rmax = outp.tile([ROWT, NTILES], mybir.dt.float32)

            # 4 row-tiles share one 4-bank PSUM tensor; a single
            # TENSOR_REDUCE with a 3D AP [128, 4, SPAN] reduces all 4
            # (out free size 4), amortizing the per-op PSUM overhead.
            # Even reduce-groups first: they only need c chunk r=0.
            g_order = list(range(0, NGROUP, 2)) + list(range(1, NGROUP, 2))
            for g in g_order:
                r = g % 2
                q_sb, qcol = q_slice(g)
                ps = psum_pool.tile(
                    [ROWT, 4, 512], mybir.dt.float32, tag="ps", name=f"ps{g}"
                )
                for j in range(4):
                    p0 = 32 * j
                    nc.tensor.matmul(
                        ps[:, j, :SPAN],
                        lhsT=q_sb[p0 : p0 + 32, qcol : qcol + ROWT],
                        rhs=c_sbs[r][p0 : p0 + 32, :],
                        start=True,
                        stop=True,
                        tile_position=(32 * j, 0),
                    )
                nc.vector.reduce_max(
                    rmax[:, 4 * g : 4 * g + 4],
                    ps[:, :, :SPAN],
                    axis=mybir.AxisListType.X,
                )

            # Stream the output in two halves (even-g columns finish first
            # under g_order) so the exit drain has less left to wait for.
            rm_v = rmax.rearrange("p (g c) -> p g c", c=4)
            ro_v = rowmax_out.rearrange("p (g c) -> p g c", c=4)
            nc.scalar.dma_start(
                out=ro_v[:, 0:NGROUP:2, :], in_=rm_v[:, 0:NGROUP:2, :]
            )
            nc.sync.dma_start(
                out=ro_v[:, 1:NGROUP:2, :], in_=rm_v[:, 1:NGROUP:2, :]
            )
    nc.compile()
    return nc


def _job_arrays(A, Bset):
    """Build per-row-group gathered feature arrays for one job."""
    ao = np.argsort(A[:, 0], kind="stable")
    bo = np.argsort(Bset[:, 0], kind="stable")
    As = np.ascontiguousarray(A[ao])
    Bs = np.ascontiguousarray(Bset[bo])

    qfeat = np.empty((4, N), np.float32)
    qfeat[0:3] = (2.0 * As).T
    qfeat[3] = 1.0

    cfeat = np.empty((4, PADDED), np.float32)
    cfeat[0:3] = PAD_COORD
    cfeat[3] = -3.0 * PAD_COORD * PAD_COORD
    cfeat[0:3, WL : WL + N] = Bs.T
    cfeat[3, WL : WL + N] = -(Bs.astype(np.float64) ** 2).sum(1).astype(np.float32)

    # c_big interleaved: partition 32j + 4m + f = (feature f, row group j,
    # chunk m), chunk m covering reduce-groups {2m, 2m+1}.
    # q_stat zero-padded stationary: for reduce-group g, row group j, the
    # [32, 128] slice at columns 128g has features only in rows 4*(g//2)..+4.
    q_stat = np.zeros((128, NGROUP * ROWT), np.float32)
    c_big = np.empty((128, 2 * SPAN), np.float32)
    g = np.arange(NGROUP)
    for j in range(4):
        t = 4 * g + j
        cidx = (ROWT * t)[:, None] + np.arange(SPAN)[None, :]
        cj = cfeat[:, cidx]  # [4f, 16g, SPAN]
        c_big[32 * j : 32 * j + 32] = (
            cj.reshape(4, 8, 2 * SPAN).transpose(1, 0, 2).reshape(32, 2 * SPAN)
        )
        for gg in range(NGROUP):
            tt = 4 * gg + j
            m = gg // 2
            q_stat[
                32 * j + 4 * m : 32 * j + 4 * m + 4,
                ROWT * gg : ROWT * gg + ROWT,
            ] = qfeat[:, ROWT * tt : ROWT * tt + ROWT]
    in_map = {"qfeat": q_stat, "cfeat": c_big}
    return As, Bs, in_map


def kernel(pred: np.ndarray, gt: np.ndarray) -> np.ndarray:
    global LAST_RESULTS
    pred = np.asarray(pred, dtype=np.float32)
    gt = np.asarray(gt, dtype=np.float32)
    assert pred.shape == (B, N, 3) and gt.shape == (B, N, 3)

    if "nc" not in _CACHE:
        _CACHE["nc"] = _build_program()
    nc = _CACHE["nc"]

    jobs = []
    in_maps = []
    for b in range(B):
        for A, Bset in ((pred[b], gt[b]), (gt[b], pred[b])):
            As, Bs, in_map = _job_arrays(A, Bset)
            jobs.append((As, Bs))
            in_maps.append(in_map)

    trace = bool(int(os.environ.get("CHAMFER_TRACE", "0")))
    bk = run_bass_kernel_spmd(nc, in_maps, list(range(8)), trace=trace)
    LAST_RESULTS = bk
    results = bk.results

    # Host: undo the rowmax formulation, certify, fix up, and average.
    total = 0.0
    i = np.arange(N)
    t = i // ROWT
    lo = ROWT * t - WL  # window start (unpadded coords, may be < 0)
    hi = ROWT * t + (SPAN - WL)  # window end (may be > N)
    for (As, Bs), r in zip(jobs, results):
        rowmax = np.asarray(r["rowmax"])  # [128, 64]
        asq = (As.astype(np.float64) ** 2).sum(1)
        d_band = asq - rowmax.T.reshape(-1).astype(np.float64)

        bx = Bs[:, 0].astype(np.float64)
        ax = As[:, 0].astype(np.float64)
        lmarg = np.where(lo >= 1, ax - bx[np.clip(lo - 1, 0, N - 1)], np.inf)
        rmarg = np.where(hi < N, bx[np.clip(hi, 0, N - 1)] - ax, np.inf)
        marg = np.minimum(lmarg, rmarg)
        ok = (marg >= 0) & (d_band <= marg * marg)
        bad = np.flatnonzero(~ok)
        if bad.size:
            Ad = As[bad].astype(np.float64)
            Bd = Bs.astype(np.float64)
            d = ((Ad[:, None, :] - Bd[None, :, :]) ** 2).sum(-1)
            d_band[bad] = d.min(1)
        total += d_band.mean()

    return np.float32(total / B)


# revision 35
# speedup vs baseline: 1.0245x; 1.0131x over previous
"""Chamfer distance loss kernel for Trainium2 (8 NeuronCores).

Strategy
--------
reference: D[i,j] = ||pred_i - gt_j||^2 ; out = mean_i min_j D + mean_j min_i D.

We decompose into 8 independent jobs (4 batches x 2 directions), one per core.
For one job (query set A, candidate set B, both of size N=8192):

  * Host sorts A and B by x-coordinate.  For query rank i, the true nearest
    neighbor is almost always within a small rank window of i in the sorted
    B order.  Each 128-row query tile t scans the candidate window
    [128t - WL, 128t + SPAN - WL) (clamped via padding), SPAN wide.
  * The kernel computes, per query row, max_j (2<a,b_j> - ||b_j||^2) over the
    window via a K=4 TensorE matmul (features [2ax,2ay,2az,1] x [bx,by,bz,-||b||^2])
    and a VectorE free-axis max-reduce.  Then
    min_j D = ||a||^2 - rowmax, computed on host.
  * Exactness certificate (host): for query i with window [lo, hi), any
    excluded candidate j < lo has |a_x - b_x[j]| >= a_x - b_x[lo-1], so if
    band_min <= (x-margin)^2 on both sides the band min is the true min.
    The few rows that fail the certificate get an exact host-side scan.

Cores: core = 2*batch + direction (0: pred->gt, 1: gt->pred).
"""

import os

import numpy as np

import concourse.bass as bass
import concourse.tile as tile
from concourse import bacc, mybir
from concourse.bass_utils import run_bass_kernel_spmd

N = 8192  # points per cloud (both pred and gt)
B = 4  # batches
ROWT = 128  # query rows per tile
NTILES = N // ROWT  # 64
SPAN = 256  # candidate window width per row tile
WL = 64  # left extension of the window
WR = SPAN - WL - ROWT  # 192: right extension
PADDED = WL + N + WR  # padded candidate count
PAD_COORD = 1000.0  # sentinel coordinate for padding (never wins a min)

_CACHE = {}

# test.py introspection: set to BassKernelResults of the last run
LAST_RESULTS = None


NGROUP = NTILES // 4  # 16 row-tiles per PE row group


def _build_program():
    nc = bacc.Bacc(
        "TRN2", target_bir_lowering=False, debug=False, num_devices=8
    )
    # Row group j (PE rows 32j..32j+4, via tile_position) handles row-tiles
    # t = 4g + j.  Both matmul operands must live at partitions 32j..32j+4,
    # so the host ships per-group gathered slices: group j's query columns
    # [4, 16*128] and its candidate windows [4, 16*SPAN] — no duplicated
    # bytes.  Four matmuls (one per group) run concurrently on the PE.
    # Matmul operands must start at partition 32j (PE row-group base), so we
    # use K=32 matmuls over the full 32-partition strip of row group j:
    #   - c_sb (moving) is interleaved: partition 32j + 4m + f = feature f,
    #     row group j, column-chunk m (chunk m covers reduce-groups 2m,2m+1).
    #     Its DMA writes all 128 partitions -> full 16-port bandwidth.
    #   - q_stat (stationary) is zero-padded per reduce-group: the [32, 128]
    #     weight slice for (g, j) has query features only in rows 4*(g//2)..+4
    #     and zeros elsewhere, so the other 7 chunks in the moving strip are
    #     multiplied away exactly.
    CCH = 2 * SPAN  # c columns per chunk
    qfeat_d = nc.declare_dram_parameter(
        "qfeat", [128, NGROUP * ROWT], mybir.dt.float32, isOutput=False
    )
    cfeat_d = nc.declare_dram_parameter(
        "cfeat", [128, CCH], mybir.dt.float32, isOutput=False
    )
    rowmax_out = nc.declare_dram_parameter(
        "rowmax", [ROWT, NTILES], mybir.dt.float32, isOutput=True
    )

    with tile.TileContext(nc) as tc:
        with (
            tc.tile_pool(name="feats", bufs=1) as feats,
            tc.tile_pool(name="psum", bufs=2, space="PSUM") as psum_pool,
            tc.tile_pool(name="outp", bufs=1) as outp,
        ):
            # Separate tiles per input chunk so Tile's dependency tracking
            # lets early matmuls start while later chunks are still in
            # flight.  c splits by parity column (r); q: small leading
            # chunks for a fast start, bigger trailing ones to keep the
            # tile/semaphore count (and the exit-drain tail) low.
            c_sbs = [
                feats.tile([128, SPAN], mybir.dt.float32, tag=f"c{r}", name=f"c{r}")
                for r in range(2)
            ]
            QQ = 2 * ROWT
            # columns (in units of QQ): tile -> q chunk bounds
            q_bounds = [(0, 1), (1, 2), (2, 4), (4, 6), (6, 8)]
            q_sbs = [
                feats.tile(
                    [128, (b - a) * QQ],
                    mybir.dt.float32,
                    tag=f"q{i}",
                    name=f"q{i}",
                )
                for i, (a, b) in enumerate(q_bounds)
            ]
            # First matmuls need c0 + q chunk 0; the scalar queue spins up
            # ~3us earlier than sync, so the critical chunks go there.
            nc.scalar.dma_start(out=c_sbs[0][:], in_=cfeat_d[:, :SPAN])
            nc.sync.dma_start(out=q_sbs[0][:], in_=qfeat_d[:, :QQ])
            nc.scalar.dma_start(out=q_sbs[1][:], in_=qfeat_d[:, QQ : 2 * QQ])
            nc.sync.dma_start(out=q_sbs[2][:], in_=qfeat_d[:, 2 * QQ : 4 * QQ])
            nc.scalar.dma_start(out=c_sbs[1][:], in_=cfeat_d[:, SPAN:])
            nc.sync.dma_start(out=q_sbs[3][:], in_=qfeat_d[:, 4 * QQ : 6 * QQ])
            nc.scalar.dma_start(out=q_sbs[4][:], in_=qfeat_d[:, 6 * QQ :])

            def q_slice(g):
                for i, (a, b) in enumerate(q_bounds):
                    if a <= g // 2 < b:
                        return q_sbs[i], ROWT * (g - 2 * a)
                raise AssertionError

            rmax = outp.tile([ROWT, NTILES], mybir.dt.float32)

            # 4 row-tiles share one 4-bank PSUM tensor; a single
            # TENSOR_REDUCE with a 3D AP [128, 4, SPAN] reduces all 4
            # (out free size 4), amortizing the per-op PSUM overhead.
            # Even reduce-groups first: they only need c chunk r=0.
            g_order = list(range(0, NGROUP, 2)) + list(range(1, NGROUP, 2))
            for g in g_order:
                r = g % 2
                q_sb, qcol = q_slice(g)
                ps = psum_pool.tile(
                    [ROWT, 4, 512], mybir.dt.float32, tag="ps", name=f"ps{g}"
                )
                for j in range(4):
                    p0 = 32 * j
                    nc.tensor.matmul(
                        ps[:, j, :SPAN],
                        lhsT=q_sb[p0 : p0 + 32, qcol : qcol + ROWT],
                        rhs=c_sbs[r][p0 : p0 + 32, :],
                        start=True,
                        stop=True,
                        tile_position=(32 * j, 0),
                    )
                nc.vector.reduce_max(
                    rmax[:, 4 * g : 4 * g + 4],
                    ps[:, :, :SPAN],
                    axis=mybir.AxisListType.X,
                )

            # Stream the output in two halves (even-g columns finish first
            # under g_order) so the exit drain has less left to wait for.
            rm_v = rmax.rearrange("p (g c) -> p g c", c=4)
            ro_v = rowmax_out.rearrange("p (g c) -> p g c", c=4)
            nc.scalar.dma_start(
                out=ro_v[:, 0:NGROUP:2, :], in_=rm_v[:, 0:NGROUP:2, :]
            )
            nc.sync.dma_start(
                out=ro_v[:, 1:NGROUP:2, :], in_=rm_v[:, 1:NGROUP:2, :]
            )
    nc.compile()
    return nc


def _job_arrays(A, Bset):
    """Build per-row-group gathered feature arrays for one job."""
    ao = np.argsort(A[:, 0], kind="stable")
    bo = np.argsort(Bset[:, 0], kind="stable")
    As = np.ascontiguousarray(A[ao])
    Bs = np.ascontiguousarray(Bset[bo])

    qfeat = np.empty((4, N), np.float32)
    qfeat[0:3] = (2.0 * As).T
    qfeat[3] = 1.0

    cfeat = np.empty((4, PADDED), np.float32)
    cfeat[0:3] = PAD_COORD
    cfeat[3] = -3.0 * PAD_COORD * PAD_COORD
    cfeat[0:3, WL : WL + N] = Bs.T
    cfeat[3, WL : WL + N] = -(Bs.astype(np.float64) ** 2).sum(1).astype(np.float32)

    # c_big interleaved: partition 32j + 4m + f = (feature f, row group j,
    # chunk m), chunk m covering reduce-groups {2m, 2m+1}.
    # q_stat zero-padded stationary: for reduce-group g, row group j, the
    # [32, 128] slice at columns 128g has features only in rows 4*(g//2)..+4.
    q_stat = np.zeros((128, NGROUP * ROWT), np.float32)
    c_big = np.empty((128, 2 * SPAN), np.float32)
    g = np.arange(NGROUP)
    for j in range(4):
        t = 4 * g + j
        cidx = (ROWT * t)[:, None] + np.arange(SPAN)[None, :]
        cj = cfeat[:, cidx]  # [4f, 16g, SPAN]
        c_big[32 * j : 32 * j + 32] = (
            cj.reshape(4, 8, 2 * SPAN).transpose(1, 0, 2).reshape(32, 2 * SPAN)
        )
        for gg in range(NGROUP):
            tt = 4 * gg + j
            m = gg // 2
            q_stat[
                32 * j + 4 * m : 32 * j + 4 * m + 4,
                ROWT * gg : ROWT * gg + ROWT,
            ] = qfeat[:, ROWT * tt : ROWT * tt + ROWT]
    in_map = {"qfeat": q_stat, "cfeat": c_big}
    return As, Bs, in_map


def kernel(pred: np.ndarray, gt: np.ndarray) -> np.ndarray:
    global LAST_RESULTS
    pred = np.asarray(pred, dtype=np.float32)
    gt = np.asarray(gt, dtype=np.float32)
    assert pred.shape == (B, N, 3) and gt.shape == (B, N, 3)

    if "nc" not in _CACHE:
        _CACHE["nc"] = _build_program()
    nc = _CACHE["nc"]

    jobs = []
    in_maps = []
    for b in range(B):
        for A, Bset in ((pred[b], gt[b]), (gt[b], pred[b])):
            As, Bs, in_map = _job_arrays(A, Bset)
            jobs.append((As, Bs))
            in_maps.append(in_map)

    trace = bool(int(os.environ.get("CHAMFER_TRACE", "0")))
    bk = run_bass_kernel_spmd(nc, in_maps, list(range(8)), trace=trace)
    LAST_RESULTS = bk
    results = bk.results

    # Host: undo the rowmax formulation, certify, fix up, and average.
    total = 0.0
    i = np.arange(N)
    t = i // ROWT
    lo = ROWT * t - WL  # window start (unpadded coords, may be < 0)
    hi = ROWT * t + (SPAN - WL)  # window end (may be > N)
    for (As, Bs), r in zip(jobs, results):
        rowmax = np.asarray(r["rowmax"])  # [128, 64]
        asq = (As.astype(np.float64) ** 2).sum(1)
        d_band = asq - rowmax.T.reshape(-1).astype(np.float64)

        bx = Bs[:, 0].astype(np.float64)
        ax = As[:, 0].astype(np.float64)
        lmarg = np.where(lo >= 1, ax - bx[np.clip(lo - 1, 0, N - 1)], np.inf)
        rmarg = np.where(hi < N, bx[np.clip(hi, 0, N - 1)] - ax, np.inf)
        marg = np.minimum(lmarg, rmarg)
        ok = (marg >= 0) & (d_band <= marg * marg)
        bad = np.flatnonzero(~ok)
        if bad.size:
            Bd = Bs.astype(np.float64)
            for s in range(0, bad.size, 256):
                idx = bad[s : s + 256]
                Ad = As[idx].astype(np.float64)
                d = ((Ad[:, None, :] - Bd[None, :, :]) ** 2).sum(-1)
                d_band[idx] = d.min(1)
        total += d_band.mean()

    return np.float32(total / B)


# revision 36
# speedup vs baseline: 1.1807x; 1.1524x over previous
"""Chamfer distance loss kernel for Trainium2 (8 NeuronCores).

Strategy
--------
reference: D[i,j] = ||pred_i - gt_j||^2 ; out = mean_i min_j D + mean_j min_i D.

We decompose into 8 independent jobs (4 batches x 2 directions), one per core.
For one job (query set A, candidate set B, both of size N=8192):

  * Host sorts A and B by x-coordinate.  For query rank i, the true nearest
    neighbor is almost always within a small rank window of i in the sorted
    B order.  Each 128-row query tile t scans the candidate window
    [128t - WL, 128t + SPAN - WL) (clamped via padding), SPAN wide.
  * The kernel computes, per query row, max_j (2<a,b_j> - ||b_j||^2) over the
    window via a K=4 TensorE matmul (features [2ax,2ay,2az,1] x [bx,by,bz,-||b||^2])
    and a VectorE free-axis max-reduce.  Then
    min_j D = ||a||^2 - rowmax, computed on host.
  * Exactness certificate (host): for query i with window [lo, hi), any
    excluded candidate j < lo has |a_x - b_x[j]| >= a_x - b_x[lo-1], so if
    band_min <= (x-margin)^2 on both sides the band min is the true min.
    The few rows that fail the certificate get an exact host-side scan.

Cores: core = 2*batch + direction (0: pred->gt, 1: gt->pred).
"""

import os

import numpy as np

import concourse.bass as bass
import concourse.tile as tile
from concourse import bacc, mybir
from concourse.bass_utils import run_bass_kernel_spmd

N = 8192  # points per cloud (both pred and gt)
B = 4  # batches
ROWT = 128  # query rows per tile
NTILES = N // ROWT  # 64
SPAN = 256  # candidate window width per row tile
WL = 64  # left extension of the window
WR = SPAN - WL - ROWT  # 192: right extension
PADDED = WL + N + WR  # padded candidate count
PAD_COORD = 1000.0  # sentinel coordinate for padding (never wins a min)

_CACHE = {}

# test.py introspection: set to BassKernelResults of the last run
LAST_RESULTS = None


NGROUP = NTILES // 4  # 16 row-tiles per PE row group


def _build_program():
    nc = bacc.Bacc(
        "TRN2", target_bir_lowering=False, debug=False, num_devices=8
    )
    # Row group j (PE rows 32j..32j+4, via tile_position) handles row-tiles
    # t = 4g + j.  Both matmul operands must live at partitions 32j..32j+4,
    # so the host ships per-group gathered slices: group j's query columns
    # [4, 16*128] and its candidate windows [4, 16*SPAN] — no duplicated
    # bytes.  Four matmuls (one per group) run concurrently on the PE.
    # Matmul operands must start at partition 32j (PE row-group base), so we
    # use K=32 matmuls over the full 32-partition strip of row group j:
    #   - c_sb (moving) is interleaved: partition 32j + 4m + f = feature f,
    #     row group j, column-chunk m (chunk m covers reduce-groups 2m,2m+1).
    #     Its DMA writes all 128 partitions -> full 16-port bandwidth.
    #   - q_stat (stationary) is zero-padded per reduce-group: the [32, 128]
    #     weight slice for (g, j) has query features only in rows 4*(g//2)..+4
    #     and zeros elsewhere, so the other 7 chunks in the moving strip are
    #     multiplied away exactly.
    CCH = 2 * SPAN  # c columns per chunk
    qfeat_d = nc.declare_dram_parameter(
        "qfeat", [128, NGROUP * ROWT], mybir.dt.float32, isOutput=False
    )
    cfeat_d = nc.declare_dram_parameter(
        "cfeat", [128, CCH], mybir.dt.float32, isOutput=False
    )
    rowmax_out = nc.declare_dram_parameter(
        "rowmax", [ROWT, NTILES], mybir.dt.float32, isOutput=True
    )

    with tile.TileContext(nc) as tc:
        with (
            tc.tile_pool(name="feats", bufs=1) as feats,
            tc.tile_pool(name="psum", bufs=2, space="PSUM") as psum_pool,
            tc.tile_pool(name="outp", bufs=1) as outp,
        ):
            # Separate tiles per input chunk so Tile's dependency tracking
            # lets early matmuls start while later chunks are still in
            # flight.  c splits by parity column (r); q: small leading
            # chunks for a fast start, bigger trailing ones to keep the
            # tile/semaphore count (and the exit-drain tail) low.
            c_sbs = [
                feats.tile([128, SPAN], mybir.dt.float32, tag=f"c{r}", name=f"c{r}")
                for r in range(2)
            ]
            QQ = 2 * ROWT
            # columns (in units of QQ): tile -> q chunk bounds
            q_bounds = [(0, 1), (1, 2), (2, 4), (4, 6), (6, 8)]
            q_sbs = [
                feats.tile(
                    [128, (b - a) * QQ],
                    mybir.dt.float32,
                    tag=f"q{i}",
                    name=f"q{i}",
                )
                for i, (a, b) in enumerate(q_bounds)
            ]
            # First matmuls need c0 + q chunk 0; the scalar queue spins up
            # ~3us earlier than sync, so the critical chunks go there.
            nc.scalar.dma_start(out=c_sbs[0][:], in_=cfeat_d[:, :SPAN])
            nc.sync.dma_start(out=q_sbs[0][:], in_=qfeat_d[:, :QQ])
            nc.scalar.dma_start(out=q_sbs[1][:], in_=qfeat_d[:, QQ : 2 * QQ])
            nc.sync.dma_start(out=q_sbs[2][:], in_=qfeat_d[:, 2 * QQ : 4 * QQ])
            nc.scalar.dma_start(out=c_sbs[1][:], in_=cfeat_d[:, SPAN:])
            nc.sync.dma_start(out=q_sbs[3][:], in_=qfeat_d[:, 4 * QQ : 6 * QQ])
            nc.scalar.dma_start(out=q_sbs[4][:], in_=qfeat_d[:, 6 * QQ :])

            def q_slice(g):
                for i, (a, b) in enumerate(q_bounds):
                    if a <= g // 2 < b:
                        return q_sbs[i], ROWT * (g - 2 * a)
                raise AssertionError

            rmax = outp.tile([ROWT, NTILES], mybir.dt.float32)

            # 4 row-tiles share one 4-bank PSUM tensor; a single
            # TENSOR_REDUCE with a 3D AP [128, 4, SPAN] reduces all 4
            # (out free size 4), amortizing the per-op PSUM overhead.
            # Even reduce-groups first: they only need c chunk r=0.
            g_order = list(range(0, NGROUP, 2)) + list(range(1, NGROUP, 2))
            for g in g_order:
                r = g % 2
                q_sb, qcol = q_slice(g)
                ps = psum_pool.tile(
                    [ROWT, 4, 512], mybir.dt.float32, tag="ps", name=f"ps{g}"
                )
                for j in range(4):
                    p0 = 32 * j
                    nc.tensor.matmul(
                        ps[:, j, :SPAN],
                        lhsT=q_sb[p0 : p0 + 32, qcol : qcol + ROWT],
                        rhs=c_sbs[r][p0 : p0 + 32, :],
                        start=True,
                        stop=True,
                        tile_position=(32 * j, 0),
                    )
                nc.vector.reduce_max(
                    rmax[:, 4 * g : 4 * g + 4],
                    ps[:, :, :SPAN],
                    axis=mybir.AxisListType.X,
                )

            nc.sync.dma_start(out=rowmax_out[:], in_=rmax[:])
    nc.compile()
    return nc


def _job_arrays(A, Bset):
    """Build per-row-group gathered feature arrays for one job."""
    ao = np.argsort(A[:, 0], kind="stable")
    bo = np.argsort(Bset[:, 0], kind="stable")
    As = np.ascontiguousarray(A[ao])
    Bs = np.ascontiguousarray(Bset[bo])

    qfeat = np.empty((4, N), np.float32)
    qfeat[0:3] = (2.0 * As).T
    qfeat[3] = 1.0

    cfeat = np.empty((4, PADDED), np.float32)
    cfeat[0:3] = PAD_COORD
    cfeat[3] = -3.0 * PAD_COORD * PAD_COORD
    cfeat[0:3, WL : WL + N] = Bs.T
    cfeat[3, WL : WL + N] = -(Bs.astype(np.float64) ** 2).sum(1).astype(np.float32)

    # c_big interleaved: partition 32j + 4m + f = (feature f, row group j,
    # chunk m), chunk m covering reduce-groups {2m, 2m+1}.
    # q_stat zero-padded stationary: for reduce-group g, row group j, the
    # [32, 128] slice at columns 128g has features only in rows 4*(g//2)..+4.
    q_stat = np.zeros((128, NGROUP * ROWT), np.float32)
    c_big = np.empty((128, 2 * SPAN), np.float32)
    g = np.arange(NGROUP)
    for j in range(4):
        t = 4 * g + j
        cidx = (ROWT * t)[:, None] + np.arange(SPAN)[None, :]
        cj = cfeat[:, cidx]  # [4f, 16g, SPAN]
        c_big[32 * j : 32 * j + 32] = (
            cj.reshape(4, 8, 2 * SPAN).transpose(1, 0, 2).reshape(32, 2 * SPAN)
        )
        for gg in range(NGROUP):
            tt = 4 * gg + j
            m = gg // 2
            q_stat[
                32 * j + 4 * m : 32 * j + 4 * m + 4,
                ROWT * gg : ROWT * gg + ROWT,
            ] = qfeat[:, ROWT * tt : ROWT * tt + ROWT]
    in_map = {"qfeat": q_stat, "cfeat": c_big}
    return As, Bs, in_map


def kernel(pred: np.ndarray, gt: np.ndarray) -> np.ndarray:
    global LAST_RESULTS
    pred = np.asarray(pred, dtype=np.float32)
    gt = np.asarray(gt, dtype=np.float32)
    assert pred.shape == (B, N, 3) and gt.shape == (B, N, 3)

    if "nc" not in _CACHE:
        _CACHE["nc"] = _build_program()
    nc = _CACHE["nc"]

    jobs = []
    in_maps = []
    for b in range(B):
        for A, Bset in ((pred[b], gt[b]), (gt[b], pred[b])):
            As, Bs, in_map = _job_arrays(A, Bset)
            jobs.append((As, Bs))
            in_maps.append(in_map)

    trace = bool(int(os.environ.get("CHAMFER_TRACE", "0")))
    bk = run_bass_kernel_spmd(nc, in_maps, list(range(8)), trace=trace)
    LAST_RESULTS = bk
    results = bk.results

    # Host: undo the rowmax formulation, certify, fix up, and average.
    total = 0.0
    i = np.arange(N)
    t = i // ROWT
    lo = ROWT * t - WL  # window start (unpadded coords, may be < 0)
    hi = ROWT * t + (SPAN - WL)  # window end (may be > N)
    for (As, Bs), r in zip(jobs, results):
        rowmax = np.asarray(r["rowmax"])  # [128, 64]
        asq = (As.astype(np.float64) ** 2).sum(1)
        d_band = asq - rowmax.T.reshape(-1).astype(np.float64)

        bx = Bs[:, 0].astype(np.float64)
        ax = As[:, 0].astype(np.float64)
        lmarg = np.where(lo >= 1, ax - bx[np.clip(lo - 1, 0, N - 1)], np.inf)
        rmarg = np.where(hi < N, bx[np.clip(hi, 0, N - 1)] - ax, np.inf)
        marg = np.minimum(lmarg, rmarg)
        ok = (marg >= 0) & (d_band <= marg * marg)
        bad = np.flatnonzero(~ok)
        if bad.size:
            Bd = Bs.astype(np.float64)
            for s in range(0, bad.size, 256):
                idx = bad[s : s + 256]
                Ad = As[idx].astype(np.float64)
                d = ((Ad[:, None, :] - Bd[None, :, :]) ** 2).sum(-1)
                d_band[idx] = d.min(1)
        total += d_band.mean()

    return np.float32(total / B)


# revision 40
# speedup vs baseline: 1.2057x; 1.0211x over previous
"""Chamfer distance loss kernel for Trainium2 (8 NeuronCores).

Strategy
--------
reference: D[i,j] = ||pred_i - gt_j||^2 ; out = mean_i min_j D + mean_j min_i D.

We decompose into 8 independent jobs (4 batches x 2 directions), one per core.
For one job (query set A, candidate set B, both of size N=8192):

  * Host sorts A and B by x-coordinate.  For query rank i, the true nearest
    neighbor is almost always within a small rank window of i in the sorted
    B order.  Each 128-row query tile t scans the candidate window
    [128t - WL, 128t + SPAN - WL) (clamped via padding), SPAN wide.
  * The kernel computes, per query row, max_j (2<a,b_j> - ||b_j||^2) over the
    window via TensorE matmuls (features [2ax,2ay,2az,1] x [bx,by,bz,-||b||^2])
    and a VectorE free-axis max-reduce.  Then
    min_j D = ||a||^2 - rowmax, computed on host.
  * Exactness certificate (host): for query i with window [lo, hi), any
    excluded candidate j < lo has |a_x - b_x[j]| >= a_x - b_x[lo-1], so if
    band_min <= (x-margin)^2 on both sides the band min is the true min.
    The rows that fail the certificate get an exact host-side scan.

Kernel-side structure (all 8 cores run the same program, SPMD):
  * Row-tile t is handled by PE row group j = t % 4 via tile_position row
    packing, so four matmuls run concurrently on the 128x128 PE array.
  * Operands must start at partition 32j, so K=32 matmuls span row group
    j's full 32-partition strip; the moving candidate windows are
    partition-interleaved across chunks and the stationary query weights
    are zero-padded so exactly one chunk contributes per matmul.  This
    keeps every input DMA 128 partitions wide (full SBUF port bandwidth).
  * One TENSOR_REDUCE with a 3D AP [128, 4, SPAN] reduces 4 row-tiles.

Cores: core = 2*batch + direction (0: pred->gt, 1: gt->pred).
"""

import os

import numpy as np

import concourse.tile as tile
from concourse import bacc, mybir
from concourse.bass_utils import run_bass_kernel_spmd

N = 8192  # points per cloud (both pred and gt)
B = 4  # batches
ROWT = 128  # query rows per tile
NTILES = N // ROWT  # 64
SPAN = 256  # candidate window width per row tile
WL = 64  # left extension of the window
WR = SPAN - WL - ROWT  # right extension
PADDED = WL + N + WR  # padded candidate count
PAD_COORD = 1000.0  # sentinel coordinate for padding (never wins a min)

_CACHE = {}

# test.py introspection: set to BassKernelResults of the last run
LAST_RESULTS = None


NGROUP = NTILES // 4  # 16 row-tiles per PE row group


def _build_program():
    nc = bacc.Bacc(
        "TRN2", target_bir_lowering=False, debug=False, num_devices=8
    )
    # Row group j (PE rows 32j.., via tile_position) handles row-tiles
    # t = 4g + j; four matmuls (one per group) run concurrently on the PE.
    # Matmul operands must start at partition 32j (PE row-group base), so we
    # use K=32 matmuls over the full 32-partition strip of row group j:
    #   - c_sb (moving) is interleaved: partition 32j + 4m + f = feature f,
    #     row group j, column-chunk m (chunk m covers reduce-groups 2m,2m+1).
    #     Its DMA writes all 128 partitions -> full 16-port bandwidth.
    #   - q_stat (stationary) is zero-padded per reduce-group: the [32, 128]
    #     weight slice for (g, j) has query features only in rows 4*(g//2)..+4
    #     and zeros elsewhere, so the other 7 chunks in the moving strip are
    #     multiplied away exactly.
    CCH = 2 * SPAN  # c columns per chunk
    qfeat_d = nc.declare_dram_parameter(
        "qfeat", [128, NGROUP * ROWT], mybir.dt.float32, isOutput=False
    )
    cfeat_d = nc.declare_dram_parameter(
        "cfeat", [128, CCH], mybir.dt.float32, isOutput=False
    )
    rowmax_out = nc.declare_dram_parameter(
        "rowmax", [ROWT, NTILES], mybir.dt.float32, isOutput=True
    )

    with tile.TileContext(nc) as tc:
        with (
            tc.tile_pool(name="feats", bufs=1) as feats,
            tc.tile_pool(name="psum", bufs=2, space="PSUM") as psum_pool,
            tc.tile_pool(name="outp", bufs=1) as outp,
        ):
            # Separate tiles per input chunk so Tile's dependency tracking
            # lets early matmuls start while later chunks are still in
            # flight.  c splits by parity column (r); q: small leading
            # chunks for a fast start, bigger trailing ones to keep the
            # tile/semaphore count (and the exit-drain tail) low.
            c_sbs = [
                feats.tile([128, SPAN], mybir.dt.float32, tag=f"c{r}", name=f"c{r}")
                for r in range(2)
            ]
            QQ = 2 * ROWT
            # columns (in units of QQ): tile -> q chunk bounds
            q_bounds = [(0, 1), (1, 2), (2, 4), (4, 6), (6, 8)]
            q_sbs = [
                feats.tile(
                    [128, (b - a) * QQ],
                    mybir.dt.float32,
                    tag=f"q{i}",
                    name=f"q{i}",
                )
                for i, (a, b) in enumerate(q_bounds)
            ]
            # First matmuls need c0 + q chunk 0; the scalar queue spins up
            # ~3us earlier than sync, so the critical chunks go there.
            nc.scalar.dma_start(out=c_sbs[0][:], in_=cfeat_d[:, :SPAN])
            nc.sync.dma_start(out=q_sbs[0][:], in_=qfeat_d[:, :QQ])
            nc.scalar.dma_start(out=q_sbs[1][:], in_=qfeat_d[:, QQ : 2 * QQ])
            nc.sync.dma_start(out=q_sbs[2][:], in_=qfeat_d[:, 2 * QQ : 4 * QQ])
            nc.scalar.dma_start(out=c_sbs[1][:], in_=cfeat_d[:, SPAN:])
            nc.sync.dma_start(out=q_sbs[3][:], in_=qfeat_d[:, 4 * QQ : 6 * QQ])
            nc.scalar.dma_start(out=q_sbs[4][:], in_=qfeat_d[:, 6 * QQ :])

            def q_slice(g):
                for i, (a, b) in enumerate(q_bounds):
                    if a <= g // 2 < b:
                        return q_sbs[i], ROWT * (g - 2 * a)
                raise AssertionError

            rmax = outp.tile([ROWT, NTILES], mybir.dt.float32)

            # 4 row-tiles share one 4-bank PSUM tensor; a single
            # TENSOR_REDUCE with a 3D AP [128, 4, SPAN] reduces all 4
            # (out free size 4), amortizing the per-op PSUM overhead.
            # Even reduce-groups first: they only need c chunk r=0.
            g_order = list(range(0, NGROUP, 2)) + list(range(1, NGROUP, 2))
            for g in g_order:
                r = g % 2
                q_sb, qcol = q_slice(g)
                ps = psum_pool.tile(
                    [ROWT, 4, 512], mybir.dt.float32, tag="ps", name=f"ps{g}"
                )
                for j in range(4):
                    p0 = 32 * j
                    nc.tensor.matmul(
                        ps[:, j, :SPAN],
                        lhsT=q_sb[p0 : p0 + 32, qcol : qcol + ROWT],
                        rhs=c_sbs[r][p0 : p0 + 32, :],
                        start=True,
                        stop=True,
                        tile_position=(32 * j, 0),
                    )
                nc.vector.reduce_max(
                    rmax[:, 4 * g : 4 * g + 4],
                    ps[:, :, :SPAN],
                    axis=mybir.AxisListType.X,
                )

            nc.sync.dma_start(out=rowmax_out[:], in_=rmax[:])
    nc.compile()
    return nc


def _job_arrays(A, Bset):
    """Build per-row-group gathered feature arrays for one job."""
    ao = np.argsort(A[:, 0], kind="stable")
    bo = np.argsort(Bset[:, 0], kind="stable")
    As = np.ascontiguousarray(A[ao])
    Bs = np.ascontiguousarray(Bset[bo])

    qfeat = np.empty((4, N), np.float32)
    qfeat[0:3] = (2.0 * As).T
    qfeat[3] = 1.0

    cfeat = np.empty((4, PADDED), np.float32)
    cfeat[0:3] = PAD_COORD
    cfeat[3] = -3.0 * PAD_COORD * PAD_COORD
    cfeat[0:3, WL : WL + N] = Bs.T
    cfeat[3, WL : WL + N] = -(Bs.astype(np.float64) ** 2).sum(1).astype(np.float32)

    # c_big interleaved: partition 32j + 4m + f = (feature f, row group j,
    # chunk m), chunk m covering reduce-groups {2m, 2m+1}.
    # q_stat zero-padded stationary: for reduce-group g, row group j, the
    # [32, 128] slice at columns 128g has features only in rows 4*(g//2)..+4.
    q_stat = np.zeros((128, NGROUP * ROWT), np.float32)
    c_big = np.empty((128, 2 * SPAN), np.float32)
    g = np.arange(NGROUP)
    for j in range(4):
        t = 4 * g + j
        cidx = (ROWT * t)[:, None] + np.arange(SPAN)[None, :]
        cj = cfeat[:, cidx]  # [4f, 16g, SPAN]
        c_big[32 * j : 32 * j + 32] = (
            cj.reshape(4, 8, 2 * SPAN).transpose(1, 0, 2).reshape(32, 2 * SPAN)
        )
        for gg in range(NGROUP):
            tt = 4 * gg + j
            m = gg // 2
            q_stat[
                32 * j + 4 * m : 32 * j + 4 * m + 4,
                ROWT * gg : ROWT * gg + ROWT,
            ] = qfeat[:, ROWT * tt : ROWT * tt + ROWT]
    in_map = {"qfeat": q_stat, "cfeat": c_big}
    return As, Bs, in_map


def kernel(pred: np.ndarray, gt: np.ndarray) -> np.ndarray:
    global LAST_RESULTS
    pred = np.asarray(pred, dtype=np.float32)
    gt = np.asarray(gt, dtype=np.float32)
    assert pred.shape == (B, N, 3) and gt.shape == (B, N, 3)

    if "nc" not in _CACHE:
        _CACHE["nc"] = _build_program()
    nc = _CACHE["nc"]

    jobs = []
    in_maps = []
    for b in range(B):
        for A, Bset in ((pred[b], gt[b]), (gt[b], pred[b])):
            As, Bs, in_map = _job_arrays(A, Bset)
            jobs.append((As, Bs))
            in_maps.append(in_map)

    trace = bool(int(os.environ.get("CHAMFER_TRACE", "0")))
    bk = run_bass_kernel_spmd(nc, in_maps, list(range(8)), trace=trace)
    LAST_RESULTS = bk
    results = bk.results

    # Host: undo the rowmax formulation, certify, fix up, and average.
    total = 0.0
    i = np.arange(N)
    t = i // ROWT
    lo = ROWT * t - WL  # window start (unpadded coords, may be < 0)
    hi = ROWT * t + (SPAN - WL)  # window end (may be > N)
    for (As, Bs), r in zip(jobs, results):
        rowmax = np.asarray(r["rowmax"])  # [128, 64]
        asq = (As.astype(np.float64) ** 2).sum(1)
        d_band = asq - rowmax.T.reshape(-1).astype(np.float64)

        bx = Bs[:, 0].astype(np.float64)
        ax = As[:, 0].astype(np.float64)
        lmarg = np.where(lo >= 1, ax - bx[np.clip(lo - 1, 0, N - 1)], np.inf)
        rmarg = np.where(hi < N, bx[np.clip(hi, 0, N - 1)] - ax, np.inf)
        marg = np.minimum(lmarg, rmarg)
        ok = (marg >= 0) & (d_band <= marg * marg)
        bad = np.flatnonzero(~ok)
        if bad.size:
            Bd = Bs.astype(np.float64)
            for s in range(0, bad.size, 256):
                idx = bad[s : s + 256]
                Ad = As[idx].astype(np.float64)
                d = ((Ad[:, None, :] - Bd[None, :, :]) ** 2).sum(-1)
                d_band[idx] = d.min(1)
        total += d_band.mean()

    return np.float32(total / B)


# revision 41
# speedup vs baseline: 1.2600x; 1.0451x over previous
"""Chamfer distance loss kernel for Trainium2 (8 NeuronCores).

Strategy
--------
reference: D[i,j] = ||pred_i - gt_j||^2 ; out = mean_i min_j D + mean_j min_i D.

We decompose into 8 independent jobs (4 batches x 2 directions), one per core.
For one job (query set A, candidate set B, both of size N=8192):

  * Host sorts A and B by x-coordinate.  For query rank i, the true nearest
    neighbor is almost always within a small rank window of i in the sorted
    B order.  Each 128-row query tile t scans the candidate window
    [128t - WL, 128t + SPAN - WL) (clamped via padding), SPAN wide.
  * The kernel computes, per query row, max_j (2<a,b_j> - ||b_j||^2) over the
    window via TensorE matmuls (features [2ax,2ay,2az,1] x [bx,by,bz,-||b||^2])
    and a VectorE free-axis max-reduce.  Then
    min_j D = ||a||^2 - rowmax, computed on host.
  * Exactness certificate (host): for query i with window [lo, hi), any
    excluded candidate j < lo has |a_x - b_x[j]| >= a_x - b_x[lo-1], so if
    band_min <= (x-margin)^2 on both sides the band min is the true min.
    The rows that fail the certificate get an exact host-side scan.

Kernel-side structure (all 8 cores run the same program, SPMD):
  * Row-tile t is handled by PE row group j = t % 4 via tile_position row
    packing, so four matmuls run concurrently on the 128x128 PE array.
  * Operands must start at partition 32j, so K=32 matmuls span row group
    j's full 32-partition strip; the moving candidate windows are
    partition-interleaved across chunks and the stationary query weights
    are zero-padded so exactly one chunk contributes per matmul.  This
    keeps every input DMA 128 partitions wide (full SBUF port bandwidth).
  * One TENSOR_REDUCE with a 3D AP [128, 4, SPAN] reduces 4 row-tiles.

Cores: core = 2*batch + direction (0: pred->gt, 1: gt->pred).
"""

import os

import numpy as np

import concourse.tile as tile
from concourse import bacc, mybir
from concourse.bass_utils import run_bass_kernel_spmd

N = 8192  # points per cloud (both pred and gt)
B = 4  # batches
ROWT = 128  # query rows per tile
NTILES = N // ROWT  # 64
SPAN = 224  # candidate window width per row tile
WL = 48  # left extension of the window
WR = SPAN - WL - ROWT  # right extension
PADDED = WL + N + WR  # padded candidate count
PAD_COORD = 1000.0  # sentinel coordinate for padding (never wins a min)

_CACHE = {}

# test.py introspection: set to BassKernelResults of the last run
LAST_RESULTS = None


NGROUP = NTILES // 4  # 16 row-tiles per PE row group


def _build_program():
    nc = bacc.Bacc(
        "TRN2", target_bir_lowering=False, debug=False, num_devices=8
    )
    # Row group j (PE rows 32j.., via tile_position) handles row-tiles
    # t = 4g + j; four matmuls (one per group) run concurrently on the PE.
    # Matmul operands must start at partition 32j (PE row-group base), so we
    # use K=32 matmuls over the full 32-partition strip of row group j:
    #   - c_sb (moving) is interleaved: partition 32j + 4m + f = feature f,
    #     row group j, column-chunk m (chunk m covers reduce-groups 2m,2m+1).
    #     Its DMA writes all 128 partitions -> full 16-port bandwidth.
    #   - q_stat (stationary) is zero-padded per reduce-group: the [32, 128]
    #     weight slice for (g, j) has query features only in rows 4*(g//2)..+4
    #     and zeros elsewhere, so the other 7 chunks in the moving strip are
    #     multiplied away exactly.
    CCH = 2 * SPAN  # c columns per chunk
    qfeat_d = nc.declare_dram_parameter(
        "qfeat", [128, NGROUP * ROWT], mybir.dt.float32, isOutput=False
    )
    cfeat_d = nc.declare_dram_parameter(
        "cfeat", [128, CCH], mybir.dt.float32, isOutput=False
    )
    rowmax_out = nc.declare_dram_parameter(
        "rowmax", [ROWT, NTILES], mybir.dt.float32, isOutput=True
    )

    with tile.TileContext(nc) as tc:
        with (
            tc.tile_pool(name="feats", bufs=1) as feats,
            tc.tile_pool(name="psum", bufs=2, space="PSUM") as psum_pool,
            tc.tile_pool(name="outp", bufs=1) as outp,
        ):
            # Separate tiles per input chunk so Tile's dependency tracking
            # lets early matmuls start while later chunks are still in
            # flight.  c splits by parity column (r); q: small leading
            # chunks for a fast start, bigger trailing ones to keep the
            # tile/semaphore count (and the exit-drain tail) low.
            c_sbs = [
                feats.tile([128, SPAN], mybir.dt.float32, tag=f"c{r}", name=f"c{r}")
                for r in range(2)
            ]
            QQ = 2 * ROWT
            # columns (in units of QQ): tile -> q chunk bounds
            q_bounds = [(0, 1), (1, 2), (2, 4), (4, 6), (6, 8)]
            q_sbs = [
                feats.tile(
                    [128, (b - a) * QQ],
                    mybir.dt.float32,
                    tag=f"q{i}",
                    name=f"q{i}",
                )
                for i, (a, b) in enumerate(q_bounds)
            ]
            # First matmuls need c0 + q chunk 0; the scalar queue spins up
            # ~3us earlier than sync, so the critical chunks go there.
            nc.scalar.dma_start(out=c_sbs[0][:], in_=cfeat_d[:, :SPAN])
            nc.sync.dma_start(out=q_sbs[0][:], in_=qfeat_d[:, :QQ])
            nc.scalar.dma_start(out=q_sbs[1][:], in_=qfeat_d[:, QQ : 2 * QQ])
            nc.sync.dma_start(out=q_sbs[2][:], in_=qfeat_d[:, 2 * QQ : 4 * QQ])
            nc.scalar.dma_start(out=c_sbs[1][:], in_=cfeat_d[:, SPAN:])
            nc.sync.dma_start(out=q_sbs[3][:], in_=qfeat_d[:, 4 * QQ : 6 * QQ])
            nc.scalar.dma_start(out=q_sbs[4][:], in_=qfeat_d[:, 6 * QQ :])

            def q_slice(g):
                for i, (a, b) in enumerate(q_bounds):
                    if a <= g // 2 < b:
                        return q_sbs[i], ROWT * (g - 2 * a)
                raise AssertionError

            rmax = outp.tile([ROWT, NTILES], mybir.dt.float32)

            # 4 row-tiles share one 4-bank PSUM tensor; a single
            # TENSOR_REDUCE with a 3D AP [128, 4, SPAN] reduces all 4
            # (out free size 4), amortizing the per-op PSUM overhead.
            # Even reduce-groups first: they only need c chunk r=0.
            g_order = list(range(0, NGROUP, 2)) + list(range(1, NGROUP, 2))
            for g in g_order:
                r = g % 2
                q_sb, qcol = q_slice(g)
                ps = psum_pool.tile(
                    [ROWT, 4, 512], mybir.dt.float32, tag="ps", name=f"ps{g}"
                )
                for j in range(4):
                    p0 = 32 * j
                    nc.tensor.matmul(
                        ps[:, j, :SPAN],
                        lhsT=q_sb[p0 : p0 + 32, qcol : qcol + ROWT],
                        rhs=c_sbs[r][p0 : p0 + 32, :],
                        start=True,
                        stop=True,
                        tile_position=(32 * j, 0),
                    )
                nc.vector.reduce_max(
                    rmax[:, 4 * g : 4 * g + 4],
                    ps[:, :, :SPAN],
                    axis=mybir.AxisListType.X,
                )

            nc.sync.dma_start(out=rowmax_out[:], in_=rmax[:])
    nc.compile()
    return nc


def _job_arrays(A, Bset):
    """Build per-row-group gathered feature arrays for one job."""
    ao = np.argsort(A[:, 0], kind="stable")
    bo = np.argsort(Bset[:, 0], kind="stable")
    As = np.ascontiguousarray(A[ao])
    Bs = np.ascontiguousarray(Bset[bo])

    qfeat = np.empty((4, N), np.float32)
    qfeat[0:3] = (2.0 * As).T
    qfeat[3] = 1.0

    cfeat = np.empty((4, PADDED), np.float32)
    cfeat[0:3] = PAD_COORD
    cfeat[3] = -3.0 * PAD_COORD * PAD_COORD
    cfeat[0:3, WL : WL + N] = Bs.T
    cfeat[3, WL : WL + N] = -(Bs.astype(np.float64) ** 2).sum(1).astype(np.float32)

    # c_big interleaved: partition 32j + 4m + f = (feature f, row group j,
    # chunk m), chunk m covering reduce-groups {2m, 2m+1}.
    # q_stat zero-padded stationary: for reduce-group g, row group j, the
    # [32, 128] slice at columns 128g has features only in rows 4*(g//2)..+4.
    q_stat = np.zeros((128, NGROUP * ROWT), np.float32)
    c_big = np.empty((128, 2 * SPAN), np.float32)
    g = np.arange(NGROUP)
    for j in range(4):
        t = 4 * g + j
        cidx = (ROWT * t)[:, None] + np.arange(SPAN)[None, :]
        cj = cfeat[:, cidx]  # [4f, 16g, SPAN]
        c_big[32 * j : 32 * j + 32] = (
            cj.reshape(4, 8, 2 * SPAN).transpose(1, 0, 2).reshape(32, 2 * SPAN)
        )
        for gg in range(NGROUP):
            tt = 4 * gg + j
            m = gg // 2
            q_stat[
                32 * j + 4 * m : 32 * j + 4 * m + 4,
                ROWT * gg : ROWT * gg + ROWT,
            ] = qfeat[:, ROWT * tt : ROWT * tt + ROWT]
    in_map = {"qfeat": q_stat, "cfeat": c_big}
    return As, Bs, in_map


def kernel(pred: np.ndarray, gt: np.ndarray) -> np.ndarray:
    global LAST_RESULTS
    pred = np.asarray(pred, dtype=np.float32)
    gt = np.asarray(gt, dtype=np.float32)
    assert pred.shape == (B, N, 3) and gt.shape == (B, N, 3)

    if "nc" not in _CACHE:
        _CACHE["nc"] = _build_program()
    nc = _CACHE["nc"]

    jobs = []
    in_maps = []
    for b in range(B):
        for A, Bset in ((pred[b], gt[b]), (gt[b], pred[b])):
            As, Bs, in_map = _job_arrays(A, Bset)
            jobs.append((As, Bs))
            in_maps.append(in_map)

    trace = bool(int(os.environ.get("CHAMFER_TRACE", "0")))
    bk = run_bass_kernel_spmd(nc, in_maps, list(range(8)), trace=trace)
    LAST_RESULTS = bk
    results = bk.results

    # Host: undo the rowmax formulation, certify, fix up, and average.
    total = 0.0
    i = np.arange(N)
    t = i // ROWT
    lo = ROWT * t - WL  # window start (unpadded coords, may be < 0)
    hi = ROWT * t + (SPAN - WL)  # window end (may be > N)
    for (As, Bs), r in zip(jobs, results):
        rowmax = np.asarray(r["rowmax"])  # [128, 64]
        asq = (As.astype(np.float64) ** 2).sum(1)
        d_band = asq - rowmax.T.reshape(-1).astype(np.float64)

        bx = Bs[:, 0].astype(np.float64)
        ax = As[:, 0].astype(np.float64)
        lmarg = np.where(lo >= 1, ax - bx[np.clip(lo - 1, 0, N - 1)], np.inf)
        rmarg = np.where(hi < N, bx[np.clip(hi, 0, N - 1)] - ax, np.inf)
        marg = np.minimum(lmarg, rmarg)
        ok = (marg >= 0) & (d_band <= marg * marg)
        bad = np.flatnonzero(~ok)
        if bad.size:
            Bd = Bs.astype(np.float64)
            for s in range(0, bad.size, 256):
                idx = bad[s : s + 256]
                Ad = As[idx].astype(np.float64)
                d = ((Ad[:, None, :] - Bd[None, :, :]) ** 2).sum(-1)
                d_band[idx] = d.min(1)
        total += d_band.mean()

    return np.float32(total / B)
